# revision 22
# baseline (speedup 1.0000x reference)
"""Trainium2 Bass kernel for the GSAT HeteroGNN problem (8 NeuronCores).

Self-contained: hardcodes shapes/sharding; only imports the concourse
toolchain.

Strategy (dst-node sharding, SPMD over 8 cores):
  - papers split into 8 chunks of 12500 (padded 12800 = 100 tiles),
    authors 8 x 6250 (padded 6400 = 50 tiles); nodes are degree-sorted
    and dealt round-robin across cores so tiles are degree-homogeneous
    and balanced across cores.
  - L1 (raw-x) aggregation: per-dst packed gather. The host packs each
    dst's <=G neighbor feature rows (fp8, 128B) into one wide table row;
    one DMA descriptor fetches up to 4KB covering several dsts, so the
    gpsimd descriptor feed (~8ns/desc single queue, ~3ns spread over 4
    SWDGE queues) is tiny.  Aggregation = DVE tree-sum + per-partition
    recip scale on ScalarE; transpose via identity-rhs matmul.
  - L2 (h1) aggregation: per-edge gather from AllGathered fp8 h1 tables
    + host-precomputed fp8 recip masks; TensorE DoubleRow fp8 matmuls
    (2 slot-columns per instruction) accumulate aggT[feat, 512] in PSUM.
  - h1 transposes for L2 root terms are per-tile identity matmuls; the
    fp8 results live in SBUF for the whole kernel (no DMA transposes).
  - AllGathers are chunked (2 for authors, 4 for papers) and aligned
    with the int16 gather bank boundaries (25600 rows) so L2 gathers of
    bank k depend only on AG chunk k.
  - gathers are spread round-robin over 4 SWDGE queues.
  - global mean-pool via ones-column matmuls accumulating in PSUM; final
    2-layer MLP on host in fp64.
"""
import os
import sys

try:
    import concourse  # noqa: F401
except ImportError:  # toolchain location in the grading container
    sys.path.insert(0, "/opt/trn_rl_repo")

import numpy as np
import ml_dtypes
from concourse import bass, bacc, mybir, tile  # noqa: F401
from concourse import bass_utils

dt = mybir.dt
F8 = ml_dtypes.float8_e4m3

# ---------------------------------------------------------------- constants
NA, NP_, E = 50000, 100000, 300000
IN, H, OUT = 128, 256, 16
C = 8                      # cores
P = 128                    # partitions
A_CAN, P_CAN = NA // C, NP_ // C              # 6250 / 12500
CHK = 3200                 # AG chunk rows per core (aligned to banks)
A_PAD = 2 * CHK            # 6400 (50 tiles, 2 AG chunks)
P_PAD = 4 * CHK            # 12800 (100 tiles, 4 AG chunks)
NA_AG, NP_AG = C * A_PAD, C * P_PAD           # 51200 / 102400
BANK = C * CHK             # 25600 rows per gather bank == AG chunk
WIN = int(os.environ.get("GNN_WIN", "2"))   # dst tiles per PSUM window
WD = WIN * P               # window width in dsts
HB = 4                     # h1 tiles batched per shard DMA write
EB = 4096                  # target gather elem bytes for L1 packed tables


class RelLayer:
    """Host-side layout for one L2 relation: slot columns per
    (window, bank), uniform across cores (max-over-cores column counts),
    int16 gather indices and fp8 recip masks."""

    def __init__(self, row_of, dstl, n_dst_pad, recip_dst_local, table_rows):
        # row_of: [C] per-edge row ids (into the AG h1 table), aligned
        # with dstl (local dst id per edge).
        self.n_tiles = n_dst_pad // P
        self.n_win = (self.n_tiles + WIN - 1) // WIN
        nb = (table_rows + BANK - 1) // BANK
        self.n_banks = nb
        self.bank_rows = BANK
        self.table_rows = table_rows

        ncols = np.zeros((self.n_win, nb), np.int64)
        per_core = []
        for c in range(C):
            rows, dl = row_of[c], dstl[c]
            w = dl // WD
            b = rows // self.bank_rows
            cnt = np.zeros((self.n_win, nb), np.int64)
            np.add.at(cnt, (w, b), 1)
            ncols = np.maximum(ncols, (cnt + P - 1) // P)
            per_core.append((rows, dl, w, b))
        self.ncols = ncols

        CAP = 16                   # max columns per gather/mask/matmul chunk
        self.col_base = np.zeros(self.n_win + 1, np.int64)
        self.ops = []              # per window: list of (bank, ioff, nidx, lcb)
        cell_base = {}
        ioff = 0
        col = 0
        for w in range(self.n_win):
            self.col_base[w] = col
            wops = []
            lcb = 0
            for b in range(nb):
                nco = int(ncols[w, b])
                if nco:
                    cell_base[(w, b)] = ioff
                    left = nco
                    while left:
                        k = min(CAP, left)
                        wops.append((b, ioff, k * P, lcb))
                        ioff += k * P // 16
                        lcb += k
                        col += k
                        left -= k
            self.ops.append(wops)
        self.col_base[self.n_win] = col
        self.total_cols = col
        self.idx_width = ioff
        self.wcols = np.diff(self.col_base).astype(np.int64)
        self.max_wcols = int(self.wcols.max()) if col else 0

        lcb_map = {}
        for w in range(self.n_win):
            for (b, io, nidx, lcb) in self.ops[w]:
                # first chunk of each (w, b) cell defines the cell layout
                if (w, b) not in lcb_map:
                    lcb_map[(w, b)] = self.col_base[w] + lcb

        self.idx16 = np.zeros((C, P, max(self.idx_width, 1)), np.int16)
        self.masks = np.zeros((C, P, max(col, 1), WD), F8)
        for c in range(C):
            rows, dl, w_e, b_e = per_core[c]
            order = np.argsort(w_e * nb + b_e, kind="stable")
            rows_s, dl_s, w_s, b_s = rows[order], dl[order], w_e[order], b_e[order]
            rec_s = recip_dst_local[c][dl_s].astype(np.float32)
            rib_s = (rows_s % self.bank_rows).astype(np.int64)
            key = w_s * nb + b_s
            cellcnt = np.bincount(key, minlength=self.n_win * nb)
            starts = np.zeros(self.n_win * nb + 1, np.int64)
            np.cumsum(cellcnt, out=starts[1:])
            j = np.arange(len(key)) - starts[key]
            flat = np.zeros(max(self.idx_width, 1) * 16, np.int16)
            iobase = np.array([cell_base.get((w, b), -1) * 16
                               for w in range(self.n_win) for b in range(nb)]
                              ).reshape(self.n_win, nb)
            pos = iobase[w_s, b_s] + j
            flat[pos] = rib_s.astype(np.int16)
            w16 = flat.reshape(-1, 16).T       # [16, width]
            self.idx16[c] = np.tile(w16, (8, 1))
            gcol = np.array([lcb_map.get((w, b), 0)
                             for w in range(self.n_win) for b in range(nb)]
                            ).reshape(self.n_win, nb)
            cc = gcol[w_s, b_s] + j // P
            pp = j % P
            off = dl_s - w_s * WD
            self.masks[c][pp, cc, off] = rec_s.astype(F8)


class PackRel:
    """Host-side per-dst packed gather tables for one L1 relation.

    One descriptor fetches elem = M*G*128 bytes: the <=G fp8 neighbor
    rows of M consecutive-tile dsts sharing a partition.  Tiers G are
    uniform across cores (max over cores per tile)."""

    def __init__(self, srcs, dstls, n_can, n_pad, recip_loc, x8pad):
        T = n_pad // P
        self.T = T
        deg = np.zeros((C, n_pad), np.int64)
        for c in range(C):
            deg[c, :n_can] = np.bincount(dstls[c], minlength=n_can)
        maxdeg_tile = deg.reshape(C, T, P).max(axis=(0, 2))
        G_t = (2 ** np.ceil(np.log2(np.maximum(maxdeg_tile, 2)))).astype(int)

        calls = []   # (t0, J, M, G, elem, base256, nidx)
        base256 = 0
        t = 0
        while t < T:
            G = int(G_t[t])
            L = 1
            while t + L < T and int(G_t[t + L]) == G:
                L += 1
            M = max(1, EB // (G * P))
            elem = M * G * P
            Jcap = max(1, 8192 // elem)
            nfull = L // M
            pos = t
            while nfull > 0:
                J = min(Jcap, nfull)
                nidx = J * P
                calls.append((pos, J, M, G, elem, base256, nidx))
                base256 += nidx * (elem // 256)
                pos += J * M
                nfull -= J
            rem = L % M
            if rem:
                elem_r = rem * G * P
                calls.append((pos, 1, rem, G, elem_r, base256, P))
                base256 += P * (elem_r // 256)
                pos += rem
            t += L
        self.calls = calls
        self.total256 = base256
        self.max_nidx = max(cc[6] for cc in calls)

        self.tables = np.zeros((C, base256, 256), F8)
        self.recip = np.ones((C, P, T), np.float32)
        Gmax = int(G_t.max())
        for c in range(C):
            order = np.argsort(dstls[c], kind="stable")
            ss = srcs[c][order]
            dst_sorted = dstls[c][order]
            cnt = np.bincount(dstls[c], minlength=n_pad)
            indptr = np.zeros(n_pad + 1, np.int64)
            np.cumsum(cnt, out=indptr[1:])
            nbr = np.full((n_pad, Gmax), -1, np.int64)
            jpos = np.arange(len(ss)) - indptr[dst_sorted]
            nbr[dst_sorted, jpos] = ss
            rl = recip_loc[c]
            self.recip[c] = np.pad(rl, (0, n_pad - len(rl)),
                                   constant_values=1.0).reshape(T, P).T
            for (t0, J, M, G, elem, b256, nidx) in calls:
                jj = np.arange(J)
                mm = np.arange(M)
                pp = np.arange(P)
                d = ((t0 + jj[:, None, None] * M + mm[None, :, None]) * P
                     + pp[None, None, :])                     # [J, M, P]
                ids = nbr[d][..., :G]                         # [J, M, P, G]
                rows = x8pad[ids + 1]                         # [J, M, P, G, 128]
                rows = rows.transpose(0, 2, 1, 3, 4)          # [J, P, M, G, 128]
                flat = rows.reshape(J * P, elem)
                self.tables[c, b256:b256 + nidx * (elem // 256)] = (
                    flat.reshape(-1, 256))


def _deal_perm(deg, n, can):
    """Degree-sorted round-robin deal: rank r -> core r%C, slot r//C."""
    order = np.argsort(-deg, kind="stable")
    perm = np.empty(n, np.int64)
    r = np.arange(n)
    perm[order] = (r % C) * can + (r // C)
    return perm


def _prep(inputs):
    f = lambda k: np.asarray(inputs[k], np.float32)
    x_author, x_paper = f("x_author"), f("x_paper")
    ws, wd = (np.asarray(inputs["ei_writes_src"], np.int64),
              np.asarray(inputs["ei_writes_dst"], np.int64))
    bs, bd = (np.asarray(inputs["ei_wb_src"], np.int64),
              np.asarray(inputs["ei_wb_dst"], np.int64))
    for k in ("c1w_bl", "c1b_bl", "c2w_bl", "c2b_bl", "skipA_b", "skipP_b",
              ):
        assert not np.any(f(k)), f"nonzero bias {k} unsupported"

    pa_perm = _deal_perm(np.bincount(wd, minlength=NP_), NP_, P_CAN)
    au_perm = _deal_perm(np.bincount(bd, minlength=NA), NA, A_CAN)
    inv_pa = np.empty(NP_, np.int64)
    inv_pa[pa_perm] = np.arange(NP_)
    inv_au = np.empty(NA, np.int64)
    inv_au[au_perm] = np.arange(NA)
    x_paper = x_paper[inv_pa]
    x_author = x_author[inv_au]
    wd, bs = pa_perm[wd], pa_perm[bs]
    ws, bd = au_perm[ws], au_perm[bd]

    cnt_p = np.bincount(wd, minlength=NP_).astype(np.float32)
    cnt_a = np.bincount(bd, minlength=NA).astype(np.float32)
    recip_p = 1.0 / np.maximum(cnt_p, 1.0)
    recip_a = 1.0 / np.maximum(cnt_a, 1.0)
    recip_p_loc = [recip_p[c * P_CAN:(c + 1) * P_CAN] for c in range(C)]
    recip_a_loc = [recip_a[c * A_CAN:(c + 1) * A_CAN] for c in range(C)]

    def split(src, dst, dst_can):
        srcs, dstls = [], []
        for c in range(C):
            m = (dst // dst_can) == c
            srcs.append(src[m])
            dstls.append((dst[m] % dst_can).astype(np.int64))
        return srcs, dstls

    w_src, w_dstl = split(ws, wd, P_CAN)     # writes: dst papers
    b_src, b_dstl = split(bs, bd, A_CAN)     # wb: dst authors

    xa8pad = np.zeros((NA + 1, IN), F8)
    xa8pad[1:] = x_author.astype(F8)
    xp8pad = np.zeros((NP_ + 1, IN), F8)
    xp8pad[1:] = x_paper.astype(F8)

    packW = PackRel(w_src, w_dstl, P_CAN, P_PAD, recip_p_loc, xa8pad)
    packB = PackRel(b_src, b_dstl, A_CAN, A_PAD, recip_a_loc, xp8pad)

    # L2 AG row mapping (chunk-major): row = k*BANK + core*CHK + loc%CHK
    agW = [((s % A_CAN) // CHK) * BANK + (s // A_CAN) * CHK
           + (s % A_CAN) % CHK for s in w_src]
    agB = [((s % P_CAN) // CHK) * BANK + (s // P_CAN) * CHK
           + (s % P_CAN) % CHK for s in b_src]
    relW2 = RelLayer(agW, w_dstl, P_PAD, recip_p_loc, NA_AG)
    relB2 = RelLayer(agB, b_dstl, A_PAD, recip_a_loc, NP_AG)

    # host-transposed fp8 x chunks (root + skip lhsT)
    xaT8 = np.zeros((C, P, A_PAD), F8)
    xpT8 = np.zeros((C, P, P_PAD), F8)
    for c in range(C):
        xaT8[c, :, :A_CAN] = x_author[c * A_CAN:(c + 1) * A_CAN].T.astype(F8)
        xpT8[c, :, :P_CAN] = x_paper[c * P_CAN:(c + 1) * P_CAN].T.astype(F8)

    # weight slab: 14 x [128, 256] fp16 (transposed: [in, out])
    wT = lambda k: f(k).T.astype(np.float16)
    slabs = [wT("c1w_Wl"), wT("c1w_Wr"), wT("c1b_Wl"), wT("c1b_Wr")]
    for k in ("c2w_Wl", "c2w_Wr", "c2b_Wl", "c2b_Wr"):
        w2 = wT(k)
        slabs += [w2[:128], w2[128:]]
    slabs += [wT("skipA_W"), wT("skipP_W")]
    wslab = np.concatenate(slabs, axis=0)          # [14*128, 256]

    ident16 = np.eye(P, dtype=np.float16)
    pool_ones = np.ones((P, 1), np.float16)

    # identity idx slab for L1 packed gathers (value i at wrapped pos)
    maxn = max(packW.max_nidx, packB.max_nidx)
    flat = np.arange(maxn, dtype=np.int16)
    ident_idx = np.tile(flat.reshape(-1, 16).T, (8, 1))   # [128, maxn//16]

    in_maps = []
    for c in range(C):
        m = dict(
            tbl_W1=packW.tables[c], tbl_B1=packB.tables[c],
            recip_W1=packW.recip[c], recip_B1=packB.recip[c],
            xaT8=xaT8[c], xpT8=xpT8[c],
            wslab=wslab, pool_ones=pool_ones, ident16=ident16,
            ident_idx=ident_idx,
            idx_W2=relW2.idx16[c], mask_W2=relW2.masks[c].reshape(P, -1),
            idx_B2=relB2.idx16[c], mask_B2=relB2.masks[c].reshape(P, -1),
        )
        in_maps.append(m)
    return dict(packW=packW, packB=packB, relW2=relW2, relB2=relB2), in_maps


def _build(st):
    packW, packB = st["packW"], st["packB"]
    relW2, relB2 = st["relW2"], st["relB2"]
    nc = bacc.Bacc("TRN2", target_bir_lowering=False, debug=False,
                   num_devices=C, num_swdge_queues=4)
    f16, f32, i16, f8 = dt.float16, dt.float32, dt.int16, dt.float8e4
    ein = lambda n, s, d: nc.dram_tensor(n, s, d, kind="ExternalInput")

    tbl_W1 = ein("tbl_W1", [packW.total256, 256], f8)
    tbl_B1 = ein("tbl_B1", [packB.total256, 256], f8)
    recip_in = {"W1": ein("recip_W1", [P, packW.T], f32),
                "B1": ein("recip_B1", [P, packB.T], f32)}
    xaT8_in = ein("xaT8", [P, A_PAD], f8)
    xpT8_in = ein("xpT8", [P, P_PAD], f8)
    wslab = ein("wslab", [14 * P, H], f16)
    pool_in = ein("pool_ones", [P, 1], f16)
    ident16_in = ein("ident16", [P, P], f16)
    maxn = max(packW.max_nidx, packB.max_nidx)
    ident_idx_in = ein("ident_idx", [P, maxn // 16], i16)
    idx_h, mask_h = {}, {}
    for nm, rl in (("W2", relW2), ("B2", relB2)):
        idx_h[nm] = ein("idx_" + nm, [P, max(rl.idx_width, 1)], i16)
        mask_h[nm] = ein("mask_" + nm, [P, max(rl.total_cols, 1) * WD], f8)

    out_pool = nc.dram_tensor("out_pool", [1, 2 * H], f32,
                              kind="ExternalOutput")

    W = {k: i for i, k in enumerate(
        ["c1w_Wl", "c1w_Wr", "c1b_Wl", "c1b_Wr",
         "c2w_Wl0", "c2w_Wl1", "c2w_Wr0", "c2w_Wr1",
         "c2b_Wl0", "c2b_Wl1", "c2b_Wr0", "c2b_Wr1",
         "skipA_W", "skipP_W"])}
    relu_f = mybir.ActivationFunctionType.Relu
    copy_f = mybir.ActivationFunctionType.Copy
    DR = mybir.MatmulPerfMode.DoubleRow
    rg = [list(range(C))]
    qrr = [0]   # SWDGE queue round-robin

    with tile.TileContext(nc) as tc:
        with tc.tile_pool(name="persist", bufs=1) as pp, \
             tc.tile_pool(name="dram", bufs=1, space="DRAM") as dp, \
             tc.tile_pool(name="work", bufs=3) as wk, \
             tc.tile_pool(name="pk", bufs=2) as pk, \
             tc.tile_pool(name="sums", bufs=2) as sm, \
             tc.tile_pool(name="msgs", bufs=3) as mp, \
             tc.tile_pool(name="maskp", bufs=2) as mk, \
             tc.tile_pool(name="hwin", bufs=2) as hw, \
             tc.tile_pool(name="psA", bufs=2, space="PSUM") as psA, \
             tc.tile_pool(name="psL", bufs=2, space="PSUM") as psL, \
             tc.tile_pool(name="psT", bufs=2, space="PSUM") as psT, \
             tc.tile_pool(name="psP", bufs=1, space="PSUM") as psP:

            # ---------------- persistent loads
            idx_t = {}
            for nm, rl in (("W2", relW2), ("B2", relB2)):
                t = pp.tile([P, max(rl.idx_width, 1)], i16, name="idx" + nm)
                nc.sync.dma_start(out=t[:], in_=idx_h[nm][:])
                idx_t[nm] = t
            iid = pp.tile([P, maxn // 16], i16, name="iid")
            nc.sync.dma_start(out=iid[:], in_=ident_idx_in[:])
            wt = pp.tile([P, 14, H], f16, name="wt", tag="wt")
            nc.sync.dma_start(out=wt[:],
                              in_=wslab[:].rearrange("(s p) d -> p s d", p=P))
            pool_t = pp.tile([P, 1], f16, name="pool_t", tag="pool_t")
            nc.sync.dma_start(out=pool_t[:], in_=pool_in[:])
            ident16 = pp.tile([P, P], f16, name="ident16", tag="ident16")
            nc.sync.dma_start(out=ident16[:], in_=ident16_in[:])
            recip_t = {}
            for nm, T in (("W1", packW.T), ("B1", packB.T)):
                t = pp.tile([P, T], f32, name="recip" + nm)
                nc.sync.dma_start(out=t[:], in_=recip_in[nm][:])
                recip_t[nm] = t
            xaT8 = pp.tile([P, A_PAD], f8, name="xaT8", tag="xaT8")
            nc.sync.dma_start(out=xaT8[:], in_=xaT8_in[:])
            xpT8 = pp.tile([P, P_PAD], f8, name="xpT8", tag="xpT8")
            nc.sync.dma_start(out=xpT8[:], in_=xpT8_in[:])

            # persistent fp8 transposed h1 tables (L2 root lhsT)
            h1aT8 = pp.tile([P, 2, A_PAD], f8, name="h1aT8", tag="h1aT8")
            h1pT8 = pp.tile([P, 2, P_PAD], f8, name="h1pT8", tag="h1pT8")

            # local h1 fp8 shards + shared AG outputs
            h1a_l8 = dp.tile([A_PAD, H], f8, name="h1a_l8", tag="h1a_l8")
            h1p_l8 = dp.tile([P_PAD, H], f8, name="h1p_l8", tag="h1p_l8")
            h1a_sh = [dp.tile([BANK, H], f8, name=f"h1a_sh{k}",
                              tag=f"h1a_sh{k}", addr_space="Shared")
                      for k in range(A_PAD // CHK)]
            h1p_sh = [dp.tile([BANK, H], f8, name=f"h1p_sh{k}",
                              tag=f"h1p_sh{k}", addr_space="Shared")
                      for k in range(P_PAD // CHK)]

            def conv1(nm, tbl, Wl, Wr, xT, h_l8, hT8):
                """L1 relation: packed per-dst gathers + DVE tree-sum."""
                pack = packW if nm == "W1" else packB
                rec = recip_t[nm]
                hbuf = None
                hbase = 0
                for (t0, J, M, G, elem, b256, nidx) in pack.calls:
                    msgs = pk.tile([P, J, elem], f8, tag="pk")
                    in_ap = tbl[b256:b256 + nidx * (elem // 256), :].rearrange(
                        "(r k) b -> r (k b)", k=elem // 256)
                    nc.gpsimd.dma_gather(
                        msgs[:], in_ap, iid[:, :nidx // 16],
                        nidx, nidx, elem, single_packet=False,
                        queue_num=qrr[0] % 4)
                    qrr[0] += 1
                    JM = J * M
                    cur = msgs[:].rearrange("p j (m g f) -> p (j m) g f",
                                            m=M, g=G)
                    g = G
                    while g > 1:
                        half = g // 2
                        nxt = sm.tile([P, JM, half, IN], f16, tag="sum")
                        nc.vector.tensor_add(out=nxt[:],
                                             in0=cur[:, :, 0:half, :],
                                             in1=cur[:, :, half:g, :])
                        cur = nxt[:]
                        g = half
                    for m in range(JM):
                        t = t0 + m
                        if hbuf is None:
                            hbuf = hw.tile([P, HB, H], f8, tag="hw")
                            hbase = t
                        mean16 = wk.tile([P, IN], f16, tag="mean")
                        nc.scalar.activation(out=mean16[:], in_=cur[:, m, 0, :],
                                             func=copy_f,
                                             scale=rec[:, t:t + 1])
                        mT = psT.tile([P, P], f32, tag="pT", space="PSUM", name="mT")
                        nc.tensor.matmul(out=mT[:], lhsT=mean16[:],
                                         rhs=ident16[:], start=True, stop=True)
                        mT_sb = wk.tile([P, P], f16, tag="mTsb")
                        nc.scalar.copy(out=mT_sb[:], in_=mT[:])
                        lin = psL.tile([P, H], f32, tag="lin", space="PSUM")
                        nc.tensor.matmul(out=lin[:], lhsT=mT_sb[:],
                                         rhs=wt[:, Wl:Wl + 1, :],
                                         start=True, stop=False)
                        nc.tensor.matmul(out=lin[:],
                                         lhsT=xT[:, t * P:(t + 1) * P],
                                         rhs=wt[:, Wr:Wr + 1, :],
                                         start=False, stop=True)
                        k = t - hbase
                        nc.scalar.activation(out=hbuf[:, k, :], in_=lin[:],
                                             func=relu_f)
                        # transpose h tile -> persistent fp8 table
                        for s in range(2):
                            hT = psT.tile([P, P], f32, tag="pT", space="PSUM", name="hT")
                            nc.tensor.matmul(out=hT[:],
                                             lhsT=hbuf[:, k, s * P:(s + 1) * P],
                                             rhs=ident16[:],
                                             start=True, stop=True)
                            nc.vector.tensor_copy(
                                out=hT8[:, s, t * P:(t + 1) * P], in_=hT[:])
                        if k == HB - 1 or t == pack.T - 1:
                            n = k + 1
                            nc.sync.dma_start(
                                out=h_l8[hbase * P:(hbase + n) * P, :
                                         ].rearrange("(t p) h -> p t h", p=P),
                                in_=hbuf[:, :n, :])
                            hbuf = None

            def conv2(nm, rl, table, Wl, Wr, rootT8, skipWi, skipT8, pool_ps,
                      ag_hooks=None):
                """L2 relation: per-edge gathers + DoubleRow mask matmuls,
                processed in <=16-column chunks to bound SBUF."""
                it = idx_t[nm]
                CAP = 16
                for w in range(rl.n_win):
                    if ag_hooks and w in ag_hooks:
                        ag_hooks[w]()
                    wc = int(rl.wcols[w])
                    cb = int(rl.col_base[w])
                    aggT8 = []
                    if wc:
                        aggs = []
                        for s in range(2):
                            aggs.append(psA.tile([P, WD], f32, tag="agg",
                                                 name="agg", space="PSUM"))
                        started = [False, False]
                        nops = len(rl.ops[w])
                        for oi, (b, ioff, nidx, lcb) in enumerate(rl.ops[w]):
                            nco = nidx // P
                            msgs = mp.tile([P, CAP, 256], f8, tag="msgs")
                            nc.gpsimd.dma_gather(
                                msgs[:, :nco, :], table[b][:],
                                it[:, ioff:ioff + nidx // 16],
                                nidx, nidx, 256, single_packet=False,
                                queue_num=qrr[0] % 4)
                            qrr[0] += 1
                            mask_t = mk.tile([P, CAP * WD], f8, tag="mask")
                            nc.sync.dma_start(
                                out=mask_t[:, :nco * WD],
                                in_=mask_h[nm][:, (cb + lcb) * WD:
                                               (cb + lcb + nco) * WD])
                            last_op = oi == nops - 1
                            np2 = nco // 2
                            for s in range(2):
                                for i in range(np2):
                                    nc.tensor.matmul(
                                        out=aggs[s][:],
                                        lhsT=msgs[:, 2 * i:2 * i + 2,
                                                  s * P:(s + 1) * P],
                                        rhs=mask_t[:, 2 * i * WD:
                                                   (2 * i + 2) * WD].rearrange(
                                            "p (k w) -> p k w", k=2),
                                        start=not started[s],
                                        stop=(last_op and nco % 2 == 0
                                              and i == np2 - 1),
                                        perf_mode=DR)
                                    started[s] = True
                                if nco % 2:
                                    nc.tensor.matmul(
                                        out=aggs[s][:],
                                        lhsT=msgs[:, nco - 1:nco,
                                                  s * P:(s + 1) * P],
                                        rhs=mask_t[:, (nco - 1) * WD:
                                                   nco * WD],
                                        start=not started[s], stop=last_op)
                                    started[s] = True
                        for s in range(2):
                            a8 = wk.tile([P, WD], f8, tag="aggT8")
                            nc.scalar.copy(out=a8[:], in_=aggs[s][:])
                            aggT8.append(a8)
                    for tl in range(min(WIN, rl.n_tiles - w * WIN)):
                        t = w * WIN + tl
                        lin = psL.tile([P, H], f32, tag="lin", space="PSUM")
                        first = True
                        if wc:
                            for s in range(2):
                                nc.tensor.matmul(
                                    out=lin[:],
                                    lhsT=aggT8[s][:, tl * P:(tl + 1) * P],
                                    rhs=wt[:, Wl[s]:Wl[s] + 1, :],
                                    start=first, stop=False)
                                first = False
                        for s in range(2):
                            nc.tensor.matmul(
                                out=lin[:],
                                lhsT=rootT8[:, s, t * P:(t + 1) * P],
                                rhs=wt[:, Wr[s]:Wr[s] + 1, :],
                                start=first, stop=False)
                            first = False
                        nc.tensor.matmul(
                            out=lin[:], lhsT=skipT8[:, t * P:(t + 1) * P],
                            rhs=wt[:, skipWi:skipWi + 1, :],
                            start=False, stop=True)
                        h16 = wk.tile([P, H], f16, tag="h16")
                        nc.scalar.activation(out=h16[:], in_=lin[:],
                                             func=relu_f)
                        nc.tensor.matmul(
                            out=pool_ps[:], lhsT=pool_t[:, 0:1],
                            rhs=h16[:], start=(t == 0),
                            stop=(t == rl.n_tiles - 1),
                            skip_group_check=True)

            # -------- L1 authors first (their AG gates layer 2 papers)
            conv1("B1", tbl_B1, W["c1b_Wl"], W["c1b_Wr"], xaT8,
                  h1a_l8, h1aT8)
            for k in range(A_PAD // CHK):
                nc.gpsimd.collective_compute(
                    "AllGather", mybir.AluOpType.bypass, replica_groups=rg,
                    ins=[h1a_l8[k * CHK:(k + 1) * CHK, :]],
                    outs=[h1a_sh[k][:]])
            conv1("W1", tbl_W1, W["c1w_Wl"], W["c1w_Wr"], xpT8,
                  h1p_l8, h1pT8)

            # -------- layer 2 (h1p AG chunks interleaved into the W2
            # gather stream so the CC transfers overlap it)
            pool_p = psP.tile([1, H], f32, name="pool_p", tag="pool_p",
                              space="PSUM")
            pool_a = psP.tile([1, H], f32, name="pool_a", tag="pool_a",
                              space="PSUM")

            def mk_ag2(k):
                def emit():
                    nc.gpsimd.collective_compute(
                        "AllGather", mybir.AluOpType.bypass,
                        replica_groups=rg,
                        ins=[h1p_l8[k * CHK:(k + 1) * CHK, :]],
                        outs=[h1p_sh[k][:]])
                return emit
            h0 = int(os.environ.get("GNN_HOOK0", "6"))
            hs = int(os.environ.get("GNN_HOOKSTEP", "6"))
            nwin_w2 = relW2.n_win
            hooks = {min(h0 + hs * k, nwin_w2 - 1): mk_ag2(k)
                     for k in range(P_PAD // CHK)}
            conv2("W2", relW2, h1a_sh, [W["c2w_Wl0"], W["c2w_Wl1"]],
                  [W["c2w_Wr0"], W["c2w_Wr1"]], h1pT8, W["skipP_W"], xpT8,
                  pool_p, ag_hooks=hooks)
            conv2("B2", relB2, h1p_sh, [W["c2b_Wl0"], W["c2b_Wl1"]],
                  [W["c2b_Wr0"], W["c2b_Wr1"]], h1aT8, W["skipA_W"], xaT8,
                  pool_a)

            pool_sb = wk.tile([1, 2 * H], f32, tag="poolout")
            nc.vector.tensor_copy(out=pool_sb[:, 0:H], in_=pool_a[:])
            nc.vector.tensor_copy(out=pool_sb[:, H:2 * H], in_=pool_p[:])
            nc.sync.dma_start(out=out_pool[:], in_=pool_sb[:])

    nc.compile()
    return nc


def kernel(**inputs):
    trace = bool(int(os.environ.get("GNN_TRACE", "0")))
    st, in_maps = _prep(inputs)
    nc = _build(st)
    res = bass_utils.run_bass_kernel_spmd(
        nc, in_maps, core_ids=list(range(C)), trace=trace)
    kernel.last_results = res

    pools = np.stack([res.results[c]["out_pool"] for c in range(C)])
    sum_a = pools[:, 0, :H].astype(np.float64).sum(axis=0)
    sum_p = pools[:, 0, H:].astype(np.float64).sum(axis=0)
    pooled = np.concatenate([sum_a / NA, sum_p / NP_])[None, :]
    W1 = np.asarray(inputs["cls_W1"], np.float64)
    b1 = np.asarray(inputs["cls_b1"], np.float64)
    W2 = np.asarray(inputs["cls_W2"], np.float64)
    b2 = np.asarray(inputs["cls_b2"], np.float64)
    h = np.maximum(pooled @ W1.T + b1, 0.0)
    out = h @ W2.T + b2
    return out.astype(np.float32)


# revision 23
# speedup vs baseline: 1.0416x; 1.0416x over previous
"""Trainium2 Bass kernel for the GSAT HeteroGNN problem (8 NeuronCores).

Self-contained: hardcodes shapes/sharding; only imports the concourse
toolchain.

Strategy (dst-node sharding, SPMD over 8 cores):
  - papers split into 8 chunks of 12500 (padded 12800 = 100 tiles),
    authors 8 x 6250 (padded 6400 = 50 tiles); nodes are degree-sorted
    and dealt round-robin across cores so tiles are degree-homogeneous
    and balanced across cores.
  - L1 (raw-x) aggregation: per-dst packed gather. The host packs each
    dst's <=G neighbor feature rows (fp8, 128B) into one wide table row;
    one DMA descriptor fetches up to 4KB covering several dsts, so the
    gpsimd descriptor feed (~8ns/desc single queue, ~3ns spread over 4
    SWDGE queues) is tiny.  Aggregation = DVE tree-sum + per-partition
    recip scale on ScalarE; transpose via identity-rhs matmul.
  - L2 (h1) aggregation: per-edge gather from AllGathered fp8 h1 tables
    + host-precomputed fp8 recip masks; TensorE DoubleRow fp8 matmuls
    (2 slot-columns per instruction) accumulate aggT[feat, 512] in PSUM.
  - h1 transposes for L2 root terms are per-tile identity matmuls; the
    fp8 results live in SBUF for the whole kernel (no DMA transposes).
  - AllGathers are chunked (2 for authors, 4 for papers) and aligned
    with the int16 gather bank boundaries (25600 rows) so L2 gathers of
    bank k depend only on AG chunk k.
  - gathers are spread round-robin over 4 SWDGE queues.
  - global mean-pool via ones-column matmuls accumulating in PSUM; final
    2-layer MLP on host in fp64.
"""
import os
import sys

try:
    import concourse  # noqa: F401
except ImportError:  # toolchain location in the grading container
    sys.path.insert(0, "/opt/trn_rl_repo")

import numpy as np
import ml_dtypes
from concourse import bass, bacc, mybir, tile  # noqa: F401
from concourse import bass_utils

dt = mybir.dt
F8 = ml_dtypes.float8_e4m3

# ---------------------------------------------------------------- constants
NA, NP_, E = 50000, 100000, 300000
IN, H, OUT = 128, 256, 16
C = 8                      # cores
P = 128                    # partitions
A_CAN, P_CAN = NA // C, NP_ // C              # 6250 / 12500
CHK = 3200                 # AG chunk rows per core (aligned to banks)
A_PAD = 2 * CHK            # 6400 (50 tiles, 2 AG chunks)
P_PAD = 4 * CHK            # 12800 (100 tiles, 4 AG chunks)
NA_AG, NP_AG = C * A_PAD, C * P_PAD           # 51200 / 102400
BANK = C * CHK             # 25600 rows per gather bank == AG chunk
WIN = int(os.environ.get("GNN_WIN", "4"))   # dst tiles per PSUM window
WD = WIN * P               # window width in dsts
HB = 4                     # h1 tiles batched per shard DMA write
EB = 4096                  # target gather elem bytes for L1 packed tables


class RelLayer:
    """Host-side layout for one L2 relation: slot columns per
    (window, bank), uniform across cores (max-over-cores column counts),
    int16 gather indices and fp8 recip masks."""

    def __init__(self, row_of, dstl, n_dst_pad, recip_dst_local, table_rows):
        # row_of: [C] per-edge row ids (into the AG h1 table), aligned
        # with dstl (local dst id per edge).
        self.n_tiles = n_dst_pad // P
        self.n_win = (self.n_tiles + WIN - 1) // WIN
        nb = (table_rows + BANK - 1) // BANK
        self.n_banks = nb
        self.bank_rows = BANK
        self.table_rows = table_rows

        ncols = np.zeros((self.n_win, nb), np.int64)
        per_core = []
        for c in range(C):
            rows, dl = row_of[c], dstl[c]
            w = dl // WD
            b = rows // self.bank_rows
            cnt = np.zeros((self.n_win, nb), np.int64)
            np.add.at(cnt, (w, b), 1)
            ncols = np.maximum(ncols, (cnt + P - 1) // P)
            per_core.append((rows, dl, w, b))
        self.ncols = ncols

        CAP = 32                   # max columns per gather/mask/matmul chunk
        self.col_base = np.zeros(self.n_win + 1, np.int64)
        self.ops = []              # per window: list of (bank, ioff, nidx, lcb)
        cell_base = {}
        ioff = 0
        col = 0
        for w in range(self.n_win):
            self.col_base[w] = col
            wops = []
            lcb = 0
            for b in range(nb):
                nco = int(ncols[w, b])
                if nco:
                    cell_base[(w, b)] = ioff
                    left = nco
                    while left:
                        k = min(CAP, left)
                        wops.append((b, ioff, k * P, lcb))
                        ioff += k * P // 16
                        lcb += k
                        col += k
                        left -= k
            self.ops.append(wops)
        self.col_base[self.n_win] = col
        self.total_cols = col
        self.idx_width = ioff
        self.wcols = np.diff(self.col_base).astype(np.int64)
        self.max_wcols = int(self.wcols.max()) if col else 0

        lcb_map = {}
        for w in range(self.n_win):
            for (b, io, nidx, lcb) in self.ops[w]:
                # first chunk of each (w, b) cell defines the cell layout
                if (w, b) not in lcb_map:
                    lcb_map[(w, b)] = self.col_base[w] + lcb

        self.idx16 = np.zeros((C, P, max(self.idx_width, 1)), np.int16)
        self.masks = np.zeros((C, P, max(col, 1), WD), F8)
        for c in range(C):
            rows, dl, w_e, b_e = per_core[c]
            order = np.argsort(w_e * nb + b_e, kind="stable")
            rows_s, dl_s, w_s, b_s = rows[order], dl[order], w_e[order], b_e[order]
            rec_s = recip_dst_local[c][dl_s].astype(np.float32)
            rib_s = (rows_s % self.bank_rows).astype(np.int64)
            key = w_s * nb + b_s
            cellcnt = np.bincount(key, minlength=self.n_win * nb)
            starts = np.zeros(self.n_win * nb + 1, np.int64)
            np.cumsum(cellcnt, out=starts[1:])
            j = np.arange(len(key)) - starts[key]
            flat = np.zeros(max(self.idx_width, 1) * 16, np.int16)
            iobase = np.array([cell_base.get((w, b), -1) * 16
                               for w in range(self.n_win) for b in range(nb)]
                              ).reshape(self.n_win, nb)
            pos = iobase[w_s, b_s] + j
            flat[pos] = rib_s.astype(np.int16)
            w16 = flat.reshape(-1, 16).T       # [16, width]
            self.idx16[c] = np.tile(w16, (8, 1))
            gcol = np.array([lcb_map.get((w, b), 0)
                             for w in range(self.n_win) for b in range(nb)]
                            ).reshape(self.n_win, nb)
            cc = gcol[w_s, b_s] + j // P
            pp = j % P
            off = dl_s - w_s * WD
            self.masks[c][pp, cc, off] = rec_s.astype(F8)


class PackRel:
    """Host-side per-dst packed gather tables for one L1 relation.

    One descriptor fetches elem = M*G*128 bytes: the <=G fp8 neighbor
    rows of M consecutive-tile dsts sharing a partition.  Tiers G are
    uniform across cores (max over cores per tile)."""

    def __init__(self, srcs, dstls, n_can, n_pad, recip_loc, x8pad):
        T = n_pad // P
        self.T = T
        deg = np.zeros((C, n_pad), np.int64)
        for c in range(C):
            deg[c, :n_can] = np.bincount(dstls[c], minlength=n_can)
        maxdeg_tile = deg.reshape(C, T, P).max(axis=(0, 2))
        G_t = (2 ** np.ceil(np.log2(np.maximum(maxdeg_tile, 2)))).astype(int)

        calls = []   # (t0, J, M, G, elem, base256, nidx)
        base256 = 0
        t = 0
        while t < T:
            G = int(G_t[t])
            L = 1
            while t + L < T and int(G_t[t + L]) == G:
                L += 1
            M = max(1, EB // (G * P))
            elem = M * G * P
            Jcap = max(1, 8192 // elem)
            nfull = L // M
            pos = t
            while nfull > 0:
                J = min(Jcap, nfull)
                nidx = J * P
                calls.append((pos, J, M, G, elem, base256, nidx))
                base256 += nidx * (elem // 256)
                pos += J * M
                nfull -= J
            rem = L % M
            if rem:
                elem_r = rem * G * P
                calls.append((pos, 1, rem, G, elem_r, base256, P))
                base256 += P * (elem_r // 256)
                pos += rem
            t += L
        self.calls = calls
        self.total256 = base256
        self.max_nidx = max(cc[6] for cc in calls)

        self.tables = np.zeros((C, base256, 256), F8)
        self.recip = np.ones((C, P, T), np.float32)
        Gmax = int(G_t.max())
        for c in range(C):
            order = np.argsort(dstls[c], kind="stable")
            ss = srcs[c][order]
            dst_sorted = dstls[c][order]
            cnt = np.bincount(dstls[c], minlength=n_pad)
            indptr = np.zeros(n_pad + 1, np.int64)
            np.cumsum(cnt, out=indptr[1:])
            nbr = np.full((n_pad, Gmax), -1, np.int64)
            jpos = np.arange(len(ss)) - indptr[dst_sorted]
            nbr[dst_sorted, jpos] = ss
            rl = recip_loc[c]
            self.recip[c] = np.pad(rl, (0, n_pad - len(rl)),
                                   constant_values=1.0).reshape(T, P).T
            for (t0, J, M, G, elem, b256, nidx) in calls:
                jj = np.arange(J)
                mm = np.arange(M)
                pp = np.arange(P)
                d = ((t0 + jj[:, None, None] * M + mm[None, :, None]) * P
                     + pp[None, None, :])                     # [J, M, P]
                ids = nbr[d][..., :G]                         # [J, M, P, G]
                rows = x8pad[ids + 1]                         # [J, M, P, G, 128]
                rows = rows.transpose(0, 2, 1, 3, 4)          # [J, P, M, G, 128]
                flat = rows.reshape(J * P, elem)
                self.tables[c, b256:b256 + nidx * (elem // 256)] = (
                    flat.reshape(-1, 256))


def _deal_perm(deg, n, can):
    """Degree-sorted round-robin deal: rank r -> core r%C, slot r//C."""
    order = np.argsort(-deg, kind="stable")
    perm = np.empty(n, np.int64)
    r = np.arange(n)
    perm[order] = (r % C) * can + (r // C)
    return perm


def _prep(inputs):
    f = lambda k: np.asarray(inputs[k], np.float32)
    x_author, x_paper = f("x_author"), f("x_paper")
    ws, wd = (np.asarray(inputs["ei_writes_src"], np.int64),
              np.asarray(inputs["ei_writes_dst"], np.int64))
    bs, bd = (np.asarray(inputs["ei_wb_src"], np.int64),
              np.asarray(inputs["ei_wb_dst"], np.int64))
    for k in ("c1w_bl", "c1b_bl", "c2w_bl", "c2b_bl", "skipA_b", "skipP_b",
              ):
        assert not np.any(f(k)), f"nonzero bias {k} unsupported"

    pa_perm = _deal_perm(np.bincount(wd, minlength=NP_), NP_, P_CAN)
    au_perm = _deal_perm(np.bincount(bd, minlength=NA), NA, A_CAN)
    inv_pa = np.empty(NP_, np.int64)
    inv_pa[pa_perm] = np.arange(NP_)
    inv_au = np.empty(NA, np.int64)
    inv_au[au_perm] = np.arange(NA)
    x_paper = x_paper[inv_pa]
    x_author = x_author[inv_au]
    wd, bs = pa_perm[wd], pa_perm[bs]
    ws, bd = au_perm[ws], au_perm[bd]

    cnt_p = np.bincount(wd, minlength=NP_).astype(np.float32)
    cnt_a = np.bincount(bd, minlength=NA).astype(np.float32)
    recip_p = 1.0 / np.maximum(cnt_p, 1.0)
    recip_a = 1.0 / np.maximum(cnt_a, 1.0)
    recip_p_loc = [recip_p[c * P_CAN:(c + 1) * P_CAN] for c in range(C)]
    recip_a_loc = [recip_a[c * A_CAN:(c + 1) * A_CAN] for c in range(C)]

    def split(src, dst, dst_can):
        srcs, dstls = [], []
        for c in range(C):
            m = (dst // dst_can) == c
            srcs.append(src[m])
            dstls.append((dst[m] % dst_can).astype(np.int64))
        return srcs, dstls

    w_src, w_dstl = split(ws, wd, P_CAN)     # writes: dst papers
    b_src, b_dstl = split(bs, bd, A_CAN)     # wb: dst authors

    xa8pad = np.zeros((NA + 1, IN), F8)
    xa8pad[1:] = x_author.astype(F8)
    xp8pad = np.zeros((NP_ + 1, IN), F8)
    xp8pad[1:] = x_paper.astype(F8)

    packW = PackRel(w_src, w_dstl, P_CAN, P_PAD, recip_p_loc, xa8pad)
    packB = PackRel(b_src, b_dstl, A_CAN, A_PAD, recip_a_loc, xp8pad)

    # L2 AG row mapping (core-major single AllGather per table)
    agW = [(s // A_CAN) * A_PAD + (s % A_CAN) for s in w_src]
    agB = [(s // P_CAN) * P_PAD + (s % P_CAN) for s in b_src]
    relW2 = RelLayer(agW, w_dstl, P_PAD, recip_p_loc, NA_AG)
    relB2 = RelLayer(agB, b_dstl, A_PAD, recip_a_loc, NP_AG)

    # host-transposed fp8 x chunks (root + skip lhsT)
    xaT8 = np.zeros((C, P, A_PAD), F8)
    xpT8 = np.zeros((C, P, P_PAD), F8)
    for c in range(C):
        xaT8[c, :, :A_CAN] = x_author[c * A_CAN:(c + 1) * A_CAN].T.astype(F8)
        xpT8[c, :, :P_CAN] = x_paper[c * P_CAN:(c + 1) * P_CAN].T.astype(F8)

    # weight slab: 14 x [128, 256] fp16 (transposed: [in, out])
    wT = lambda k: f(k).T.astype(np.float16)
    slabs = [wT("c1w_Wl"), wT("c1w_Wr"), wT("c1b_Wl"), wT("c1b_Wr")]
    for k in ("c2w_Wl", "c2w_Wr", "c2b_Wl", "c2b_Wr"):
        w2 = wT(k)
        slabs += [w2[:128], w2[128:]]
    slabs += [wT("skipA_W"), wT("skipP_W")]
    wslab = np.concatenate(slabs, axis=0)          # [14*128, 256]

    ident16 = np.eye(P, dtype=np.float16)
    pool_ones = np.ones((P, 1), np.float16)

    # identity idx slab for L1 packed gathers (value i at wrapped pos)
    maxn = max(packW.max_nidx, packB.max_nidx)
    flat = np.arange(maxn, dtype=np.int16)
    ident_idx = np.tile(flat.reshape(-1, 16).T, (8, 1))   # [128, maxn//16]

    in_maps = []
    for c in range(C):
        m = dict(
            tbl_W1=packW.tables[c], tbl_B1=packB.tables[c],
            recip_W1=packW.recip[c], recip_B1=packB.recip[c],
            xaT8=xaT8[c], xpT8=xpT8[c],
            wslab=wslab, pool_ones=pool_ones, ident16=ident16,
            ident_idx=ident_idx,
            idx_W2=relW2.idx16[c], mask_W2=relW2.masks[c].reshape(P, -1),
            idx_B2=relB2.idx16[c], mask_B2=relB2.masks[c].reshape(P, -1),
        )
        in_maps.append(m)
    return dict(packW=packW, packB=packB, relW2=relW2, relB2=relB2), in_maps


def _build(st):
    packW, packB = st["packW"], st["packB"]
    relW2, relB2 = st["relW2"], st["relB2"]
    nc = bacc.Bacc("TRN2", target_bir_lowering=False, debug=False,
                   num_devices=C, num_swdge_queues=4)
    f16, f32, i16, f8 = dt.float16, dt.float32, dt.int16, dt.float8e4
    ein = lambda n, s, d: nc.dram_tensor(n, s, d, kind="ExternalInput")

    tbl_W1 = ein("tbl_W1", [packW.total256, 256], f8)
    tbl_B1 = ein("tbl_B1", [packB.total256, 256], f8)
    recip_in = {"W1": ein("recip_W1", [P, packW.T], f32),
                "B1": ein("recip_B1", [P, packB.T], f32)}
    xaT8_in = ein("xaT8", [P, A_PAD], f8)
    xpT8_in = ein("xpT8", [P, P_PAD], f8)
    wslab = ein("wslab", [14 * P, H], f16)
    pool_in = ein("pool_ones", [P, 1], f16)
    ident16_in = ein("ident16", [P, P], f16)
    maxn = max(packW.max_nidx, packB.max_nidx)
    ident_idx_in = ein("ident_idx", [P, maxn // 16], i16)
    idx_h, mask_h = {}, {}
    for nm, rl in (("W2", relW2), ("B2", relB2)):
        idx_h[nm] = ein("idx_" + nm, [P, max(rl.idx_width, 1)], i16)
        mask_h[nm] = ein("mask_" + nm, [P, max(rl.total_cols, 1) * WD], f8)

    out_pool = nc.dram_tensor("out_pool", [1, 2 * H], f32,
                              kind="ExternalOutput")

    W = {k: i for i, k in enumerate(
        ["c1w_Wl", "c1w_Wr", "c1b_Wl", "c1b_Wr",
         "c2w_Wl0", "c2w_Wl1", "c2w_Wr0", "c2w_Wr1",
         "c2b_Wl0", "c2b_Wl1", "c2b_Wr0", "c2b_Wr1",
         "skipA_W", "skipP_W"])}
    relu_f = mybir.ActivationFunctionType.Relu
    copy_f = mybir.ActivationFunctionType.Copy
    DR = mybir.MatmulPerfMode.DoubleRow
    rg = [list(range(C))]
    qrr = [0]   # SWDGE queue round-robin

    with tile.TileContext(nc) as tc:
        with tc.tile_pool(name="persist", bufs=1) as pp, \
             tc.tile_pool(name="dram", bufs=1, space="DRAM") as dp, \
             tc.tile_pool(name="work", bufs=3) as wk, \
             tc.tile_pool(name="pk", bufs=2) as pk, \
             tc.tile_pool(name="sums", bufs=2) as sm, \
             tc.tile_pool(name="msgs", bufs=3) as mp, \
             tc.tile_pool(name="maskp", bufs=2) as mk, \
             tc.tile_pool(name="hwin", bufs=2) as hw, \
             tc.tile_pool(name="psA", bufs=2, space="PSUM") as psA, \
             tc.tile_pool(name="psL", bufs=2, space="PSUM") as psL, \
             tc.tile_pool(name="psT", bufs=2, space="PSUM") as psT, \
             tc.tile_pool(name="psP", bufs=1, space="PSUM") as psP:

            # ---------------- persistent loads
            idx_t = {}
            for nm, rl in (("W2", relW2), ("B2", relB2)):
                t = pp.tile([P, max(rl.idx_width, 1)], i16, name="idx" + nm)
                nc.sync.dma_start(out=t[:], in_=idx_h[nm][:])
                idx_t[nm] = t
            iid = pp.tile([P, maxn // 16], i16, name="iid")
            nc.sync.dma_start(out=iid[:], in_=ident_idx_in[:])
            wt = pp.tile([P, 14, H], f16, name="wt", tag="wt")
            nc.sync.dma_start(out=wt[:],
                              in_=wslab[:].rearrange("(s p) d -> p s d", p=P))
            pool_t = pp.tile([P, 1], f16, name="pool_t", tag="pool_t")
            nc.sync.dma_start(out=pool_t[:], in_=pool_in[:])
            ident16 = pp.tile([P, P], f16, name="ident16", tag="ident16")
            nc.sync.dma_start(out=ident16[:], in_=ident16_in[:])
            recip_t = {}
            for nm, T in (("W1", packW.T), ("B1", packB.T)):
                t = pp.tile([P, T], f32, name="recip" + nm)
                nc.sync.dma_start(out=t[:], in_=recip_in[nm][:])
                recip_t[nm] = t
            xaT8 = pp.tile([P, A_PAD], f8, name="xaT8", tag="xaT8")
            nc.sync.dma_start(out=xaT8[:], in_=xaT8_in[:])
            xpT8 = pp.tile([P, P_PAD], f8, name="xpT8", tag="xpT8")
            nc.sync.dma_start(out=xpT8[:], in_=xpT8_in[:])

            # persistent fp8 transposed h1 tables (L2 root lhsT)
            h1aT8 = pp.tile([P, 2, A_PAD], f8, name="h1aT8", tag="h1aT8")
            h1pT8 = pp.tile([P, 2, P_PAD], f8, name="h1pT8", tag="h1pT8")

            # local h1 fp8 shards + shared AG outputs
            h1a_l8 = dp.tile([A_PAD, H], f8, name="h1a_l8", tag="h1a_l8")
            h1p_l8 = dp.tile([P_PAD, H], f8, name="h1p_l8", tag="h1p_l8")
            h1a_sh = dp.tile([NA_AG, H], f8, name="h1a_sh", tag="h1a_sh",
                             addr_space="Shared")
            h1p_sh = dp.tile([NP_AG, H], f8, name="h1p_sh", tag="h1p_sh",
                             addr_space="Shared")

            def conv1(nm, tbl, Wl, Wr, xT, h_l8, hT8):
                """L1 relation: packed per-dst gathers + DVE tree-sum."""
                pack = packW if nm == "W1" else packB
                rec = recip_t[nm]
                hbuf = None
                hbase = 0
                for (t0, J, M, G, elem, b256, nidx) in pack.calls:
                    msgs = pk.tile([P, J, elem], f8, tag="pk")
                    in_ap = tbl[b256:b256 + nidx * (elem // 256), :].rearrange(
                        "(r k) b -> r (k b)", k=elem // 256)
                    nc.gpsimd.dma_gather(
                        msgs[:], in_ap, iid[:, :nidx // 16],
                        nidx, nidx, elem, single_packet=False,
                        queue_num=qrr[0] % 4)
                    qrr[0] += 1
                    JM = J * M
                    cur = msgs[:].rearrange("p j (m g f) -> p (j m) g f",
                                            m=M, g=G)
                    g = G
                    while g > 1:
                        half = g // 2
                        nxt = sm.tile([P, JM, half, IN], f16, tag="sum")
                        nc.vector.tensor_add(out=nxt[:],
                                             in0=cur[:, :, 0:half, :],
                                             in1=cur[:, :, half:g, :])
                        cur = nxt[:]
                        g = half
                    for m in range(JM):
                        t = t0 + m
                        if hbuf is None:
                            hbuf = hw.tile([P, HB, H], f8, tag="hw")
                            hbase = t
                        mean16 = wk.tile([P, IN], f16, tag="mean")
                        nc.scalar.activation(out=mean16[:], in_=cur[:, m, 0, :],
                                             func=copy_f,
                                             scale=rec[:, t:t + 1])
                        mT = psT.tile([P, P], f32, tag="pT", space="PSUM", name="mT")
                        nc.tensor.matmul(out=mT[:], lhsT=mean16[:],
                                         rhs=ident16[:], start=True, stop=True)
                        mT_sb = wk.tile([P, P], f16, tag="mTsb")
                        nc.scalar.copy(out=mT_sb[:], in_=mT[:])
                        lin = psL.tile([P, H], f32, tag="lin", space="PSUM")
                        nc.tensor.matmul(out=lin[:], lhsT=mT_sb[:],
                                         rhs=wt[:, Wl:Wl + 1, :],
                                         start=True, stop=False)
                        nc.tensor.matmul(out=lin[:],
                                         lhsT=xT[:, t * P:(t + 1) * P],
                                         rhs=wt[:, Wr:Wr + 1, :],
                                         start=False, stop=True)
                        k = t - hbase
                        nc.scalar.activation(out=hbuf[:, k, :], in_=lin[:],
                                             func=relu_f)
                        # transpose h tile -> persistent fp8 table
                        for s in range(2):
                            hT = psT.tile([P, P], f32, tag="pT", space="PSUM", name="hT")
                            nc.tensor.matmul(out=hT[:],
                                             lhsT=hbuf[:, k, s * P:(s + 1) * P],
                                             rhs=ident16[:],
                                             start=True, stop=True)
                            nc.vector.tensor_copy(
                                out=hT8[:, s, t * P:(t + 1) * P], in_=hT[:])
                        if k == HB - 1 or t == pack.T - 1:
                            n = k + 1
                            nc.sync.dma_start(
                                out=h_l8[hbase * P:(hbase + n) * P, :
                                         ].rearrange("(t p) h -> p t h", p=P),
                                in_=hbuf[:, :n, :])
                            hbuf = None

            def conv2(nm, rl, table, Wl, Wr, rootT8, skipWi, skipT8, pool_ps,
                      ag_hooks=None):
                """L2 relation: per-edge gathers + DoubleRow mask matmuls,
                processed in <=16-column chunks to bound SBUF."""
                it = idx_t[nm]
                CAP = 32
                for w in range(rl.n_win):
                    if ag_hooks and w in ag_hooks:
                        ag_hooks[w]()
                    wc = int(rl.wcols[w])
                    cb = int(rl.col_base[w])
                    aggT8 = []
                    if wc:
                        aggs = []
                        for s in range(2):
                            aggs.append(psA.tile([P, WD], f32, tag="agg",
                                                 name="agg", space="PSUM"))
                        started = [False, False]
                        nops = len(rl.ops[w])
                        for oi, (b, ioff, nidx, lcb) in enumerate(rl.ops[w]):
                            nco = nidx // P
                            b0 = b * rl.bank_rows
                            b1 = min(b0 + rl.bank_rows, rl.table_rows)
                            msgs = mp.tile([P, CAP, 256], f8, tag="msgs")
                            nc.gpsimd.dma_gather(
                                msgs[:, :nco, :], table[b0:b1, :],
                                it[:, ioff:ioff + nidx // 16],
                                nidx, nidx, 256, single_packet=False,
                                queue_num=qrr[0] % 4)
                            qrr[0] += 1
                            mask_t = mk.tile([P, CAP * WD], f8, tag="mask")
                            nc.sync.dma_start(
                                out=mask_t[:, :nco * WD],
                                in_=mask_h[nm][:, (cb + lcb) * WD:
                                               (cb + lcb + nco) * WD])
                            last_op = oi == nops - 1
                            np2 = nco // 2
                            for s in range(2):
                                for i in range(np2):
                                    nc.tensor.matmul(
                                        out=aggs[s][:],
                                        lhsT=msgs[:, 2 * i:2 * i + 2,
                                                  s * P:(s + 1) * P],
                                        rhs=mask_t[:, 2 * i * WD:
                                                   (2 * i + 2) * WD].rearrange(
                                            "p (k w) -> p k w", k=2),
                                        start=not started[s],
                                        stop=(last_op and nco % 2 == 0
                                              and i == np2 - 1),
                                        perf_mode=DR)
                                    started[s] = True
                                if nco % 2:
                                    nc.tensor.matmul(
                                        out=aggs[s][:],
                                        lhsT=msgs[:, nco - 1:nco,
                                                  s * P:(s + 1) * P],
                                        rhs=mask_t[:, (nco - 1) * WD:
                                                   nco * WD],
                                        start=not started[s], stop=last_op)
                                    started[s] = True
                        for s in range(2):
                            a8 = wk.tile([P, WD], f8, tag="aggT8")
                            nc.scalar.copy(out=a8[:], in_=aggs[s][:])
                            aggT8.append(a8)
                    for tl in range(min(WIN, rl.n_tiles - w * WIN)):
                        t = w * WIN + tl
                        lin = psL.tile([P, H], f32, tag="lin", space="PSUM")
                        first = True
                        if wc:
                            for s in range(2):
                                nc.tensor.matmul(
                                    out=lin[:],
                                    lhsT=aggT8[s][:, tl * P:(tl + 1) * P],
                                    rhs=wt[:, Wl[s]:Wl[s] + 1, :],
                                    start=first, stop=False)
                                first = False
                        for s in range(2):
                            nc.tensor.matmul(
                                out=lin[:],
                                lhsT=rootT8[:, s, t * P:(t + 1) * P],
                                rhs=wt[:, Wr[s]:Wr[s] + 1, :],
                                start=first, stop=False)
                            first = False
                        nc.tensor.matmul(
                            out=lin[:], lhsT=skipT8[:, t * P:(t + 1) * P],
                            rhs=wt[:, skipWi:skipWi + 1, :],
                            start=False, stop=True)
                        h16 = wk.tile([P, H], f16, tag="h16")
                        nc.scalar.activation(out=h16[:], in_=lin[:],
                                             func=relu_f)
                        nc.tensor.matmul(
                            out=pool_ps[:], lhsT=pool_t[:, 0:1],
                            rhs=h16[:], start=(t == 0),
                            stop=(t == rl.n_tiles - 1),
                            skip_group_check=True)

            # -------- L1 authors first (their AG gates layer 2 papers)
            conv1("B1", tbl_B1, W["c1b_Wl"], W["c1b_Wr"], xaT8,
                  h1a_l8, h1aT8)
            nc.gpsimd.collective_compute(
                "AllGather", mybir.AluOpType.bypass, replica_groups=rg,
                ins=[h1a_l8.opt()], outs=[h1a_sh.opt()])
            conv1("W1", tbl_W1, W["c1w_Wl"], W["c1w_Wr"], xpT8,
                  h1p_l8, h1pT8)

            # -------- layer 2 (h1p AG chunks interleaved into the W2
            # gather stream so the CC transfers overlap it)
            pool_p = psP.tile([1, H], f32, name="pool_p", tag="pool_p",
                              space="PSUM")
            pool_a = psP.tile([1, H], f32, name="pool_a", tag="pool_a",
                              space="PSUM")

            def ag2_emit():
                nc.gpsimd.collective_compute(
                    "AllGather", mybir.AluOpType.bypass, replica_groups=rg,
                    ins=[h1p_l8.opt()], outs=[h1p_sh.opt()])
            h0 = int(os.environ.get("GNN_HOOK0", "10"))
            hooks = {min(h0, relW2.n_win - 1): ag2_emit}
            conv2("W2", relW2, h1a_sh, [W["c2w_Wl0"], W["c2w_Wl1"]],
                  [W["c2w_Wr0"], W["c2w_Wr1"]], h1pT8, W["skipP_W"], xpT8,
                  pool_p, ag_hooks=hooks)
            conv2("B2", relB2, h1p_sh, [W["c2b_Wl0"], W["c2b_Wl1"]],
                  [W["c2b_Wr0"], W["c2b_Wr1"]], h1aT8, W["skipA_W"], xaT8,
                  pool_a)

            pool_sb = wk.tile([1, 2 * H], f32, tag="poolout")
            nc.vector.tensor_copy(out=pool_sb[:, 0:H], in_=pool_a[:])
            nc.vector.tensor_copy(out=pool_sb[:, H:2 * H], in_=pool_p[:])
            nc.sync.dma_start(out=out_pool[:], in_=pool_sb[:])

    nc.compile()
    return nc


def kernel(**inputs):
    trace = bool(int(os.environ.get("GNN_TRACE", "0")))
    st, in_maps = _prep(inputs)
    nc = _build(st)
    res = bass_utils.run_bass_kernel_spmd(
        nc, in_maps, core_ids=list(range(C)), trace=trace)
    kernel.last_results = res

    pools = np.stack([res.results[c]["out_pool"] for c in range(C)])
    sum_a = pools[:, 0, :H].astype(np.float64).sum(axis=0)
    sum_p = pools[:, 0, H:].astype(np.float64).sum(axis=0)
    pooled = np.concatenate([sum_a / NA, sum_p / NP_])[None, :]
    W1 = np.asarray(inputs["cls_W1"], np.float64)
    b1 = np.asarray(inputs["cls_b1"], np.float64)
    W2 = np.asarray(inputs["cls_W2"], np.float64)
    b2 = np.asarray(inputs["cls_b2"], np.float64)
    h = np.maximum(pooled @ W1.T + b1, 0.0)
    out = h @ W2.T + b2
    return out.astype(np.float32)
